# revision 9
# baseline (speedup 1.0000x reference)
"""Chamfer loss (brute-force, no sigma) on 8 trn2 NeuronCores.

Strategy (data-parallel over batch, one batch element per core):
  sq[m,n] = |src_m - dst_n|^2 is produced by ONE augmented matmul per tile:
     sq = L^T @ R,  K = 18 rows:
       rows 0-11 : exact 2-term bf16 split of -2*src_c x dst_c  (hi/lo cross terms)
       rows 12-14: ones (x) 3-term bf16 split of |dst_n|^2
       rows 15-17: 3-term bf16 split of |src_m|^2 (x) ones
  PE accumulates in fp32, so the full squared distance (small, >=0) is formed
  before any rounding.  ScalarE evacuates each PSUM tile to SBUF as bf16;
  VectorE then does a fused (elementwise-min + row-min-reduce) for the
  src->dst direction and a running elementwise min for the dst->src
  direction.  The dst->src partition-axis min is finished with a 32x32
  stream transpose + reduce + two cross-quadrant folds.
  Host side only shards/preps inputs and takes sqrt/mean of the 2*4096
  per-core minima.
"""

import numpy as np
import ml_dtypes
from contextlib import ExitStack

B, C = 8, 3
M = N = 4096
NCORES = 8
PB = 128          # output partition block (m rows per matmul)
KAUG = 18         # augmented contraction dim
BIG = 1.0e4       # > max possible squared distance (~150)
MMN = 512         # matmul moving free dim (one fp32 PSUM bank)

bf16np = ml_dtypes.bfloat16

# Toggles (fallbacks in case an op is unsupported somewhere in the stack)
USE_TTR = True        # fused tensor_tensor_reduce for the forward row-min
XPART_SLICE = False   # walrus rejects TT with mismatched SB base partitions


# ----------------------------------------------------------------------------
# Device program
# ----------------------------------------------------------------------------

def _body(ctx, tc, lhs, rhs, rowmin_d, colmin_d, m, n):
    import concourse.mybir as mybir

    nc = tc.nc
    f32 = mybir.dt.float32
    bf16 = mybir.dt.bfloat16
    MIN = mybir.AluOpType.min
    AX = mybir.AxisListType.X

    nblk = m // PB
    half = n // 2
    mmn = MMN if half % MMN == 0 else half
    nq = half // mmn
    nb32 = half // 32

    cpool = ctx.enter_context(tc.tile_pool(name="const", bufs=1))
    ppool = ctx.enter_context(tc.tile_pool(name="psum", bufs=2, space="PSUM"))
    spool = ctx.enter_context(tc.tile_pool(name="sb", bufs=3))
    rpool = ctx.enter_context(tc.tile_pool(name="scr", bufs=2))

    lhs_t = cpool.tile([KAUG, m], bf16)
    nc.sync.dma_start(out=lhs_t[:], in_=lhs[:])
    rhs_t = cpool.tile([KAUG, n], bf16)
    nc.sync.dma_start(out=rhs_t[:], in_=rhs[:])

    rowmin_t = cpool.tile([PB, nblk], f32)
    rm2 = cpool.tile([PB, 2 * nblk], f32)
    baccs = []
    for g in range(2):
        bacc = cpool.tile([PB, half], bf16, tag=f"bacc{g}")
        nc.vector.memset(bacc[:], BIG)
        baccs.append(bacc)

    for i in range(nblk):
        for h in range(2):
            pt = ppool.tile([PB, half], f32, tag="pt")
            for q in range(nq):
                nc.tensor.matmul(
                    pt[:, q * mmn:(q + 1) * mmn],
                    lhs_t[:, i * PB:(i + 1) * PB],
                    rhs_t[:, h * half + q * mmn: h * half + (q + 1) * mmn],
                    start=True, stop=True,
                )
            sb = spool.tile([PB, half], bf16, tag="sb")
            nc.scalar.copy(sb[:], pt[:])

            # fused elementwise no-op + free-dim min-reduce into rm2 column
            scr = rpool.tile([PB, half], bf16, tag="scr")
            nc.vector.tensor_scalar(scr[:], sb[:], float(BIG), None, MIN, MIN,
                                    accum_out=rm2[:, 2 * i + h: 2 * i + h + 1])
            nc.vector.tensor_tensor(baccs[h][:], baccs[h][:], sb[:], MIN)

    nc.vector.tensor_reduce(
        rowmin_t[:], rm2[:].rearrange("p (i h) -> p i h", h=2), AX, MIN)
    nc.sync.dma_start(out=rowmin_d[:], in_=rowmin_t[:])

    # dst->src direction: partition-axis min of bacc[p, nn] over p.
    colmin_sb = cpool.tile([32, n // 32], f32)
    for g in range(2):
        tr = rpool.tile([PB, half], bf16, tag="tr")
        nc.vector.transpose(tr[:], baccs[g][:])
        red = rpool.tile([PB, nb32], f32, tag="red")
        nc.vector.tensor_reduce(
            red[:], tr[:].rearrange("p (b i) -> p b i", i=32), AX, MIN)
        if XPART_SLICE:
            t1 = rpool.tile([64, nb32], f32, tag="t1")
            nc.vector.tensor_tensor(t1[:], red[0:64, :], red[64:128, :], MIN)
            nc.vector.tensor_tensor(colmin_sb[:, g * nb32:(g + 1) * nb32],
                                    t1[0:32, :], t1[32:64, :], MIN)
        else:
            hi = rpool.tile([64, nb32], f32, tag="hi")
            nc.sync.dma_start(out=hi[:], in_=red[64:128, :])
            t1 = rpool.tile([64, nb32], f32, tag="t1")
            nc.vector.tensor_tensor(t1[:], red[0:64, :], hi[:], MIN)
            hi2 = rpool.tile([32, nb32], f32, tag="hi2")
            nc.sync.dma_start(out=hi2[:], in_=t1[32:64, :])
            nc.vector.tensor_tensor(colmin_sb[:, g * nb32:(g + 1) * nb32],
                                    t1[0:32, :], hi2[:], MIN)
    nc.sync.dma_start(out=colmin_d[:], in_=colmin_sb[:])


def build_nc(m=M, n=N):
    import concourse.tile as tile
    import concourse.bacc as bacc
    import concourse.mybir as mybir

    f32 = mybir.dt.float32
    bf16 = mybir.dt.bfloat16
    nblk = m // PB

    nc = bacc.Bacc("TRN2", target_bir_lowering=False, debug=False)
    lhs = nc.dram_tensor("lhs_aug", [KAUG, m], bf16, kind="ExternalInput").ap()
    rhs = nc.dram_tensor("rhs_aug", [KAUG, n], bf16, kind="ExternalInput").ap()
    rowmin_d = nc.dram_tensor("rowmin", [PB, nblk], f32,
                              kind="ExternalOutput").ap()
    colmin_d = nc.dram_tensor("colmin", [32, n // 32], f32,
                              kind="ExternalOutput").ap()
    with tile.TileContext(nc) as tc:
        with ExitStack() as ctx:
            _body(ctx, tc, lhs, rhs, rowmin_d, colmin_d, m, n)
    nc.compile()
    return nc


# ----------------------------------------------------------------------------
# Host-side input prep: exact bf16 splits for the augmented operands
# ----------------------------------------------------------------------------

def _split2(x):
    """x (f64) -> (hi, lo) bf16 values returned as exact f64."""
    hi = x.astype(bf16np).astype(np.float64)
    lo = (x - hi).astype(bf16np).astype(np.float64)
    return hi, lo


def _split3(x):
    h = x.astype(bf16np).astype(np.float64)
    r = x - h
    mdl = r.astype(bf16np).astype(np.float64)
    l = (r - mdl).astype(bf16np).astype(np.float64)
    return h, mdl, l


def prep_inputs(pc_src, pc_dst):
    """Build per-batch augmented operands L, R: [B, 18, M/N] bf16."""
    s = np.asarray(pc_src, dtype=np.float64)   # [B, 3, M]
    d = np.asarray(pc_dst, dtype=np.float64)   # [B, 3, N]
    b, c, m = s.shape
    n = d.shape[2]

    s_hi, s_lo = _split2(s)
    d_hi, d_lo = _split2(d)
    s_eff = s_hi + s_lo
    d_eff = d_hi + d_lo
    s2 = (s_eff ** 2).sum(axis=1)              # [B, M]
    d2 = (d_eff ** 2).sum(axis=1)              # [B, N]
    s2h, s2m, s2l = _split3(s2)
    d2h, d2m, d2l = _split3(d2)

    L = np.zeros((b, KAUG, m), dtype=np.float64)
    R = np.zeros((b, KAUG, n), dtype=np.float64)
    L[:, 0:3] = -2.0 * s_hi
    R[:, 0:3] = d_hi
    L[:, 3:6] = -2.0 * s_hi
    R[:, 3:6] = d_lo
    L[:, 6:9] = -2.0 * s_lo
    R[:, 6:9] = d_hi
    L[:, 9:12] = -2.0 * s_lo
    R[:, 9:12] = d_lo
    L[:, 12:15] = 1.0
    R[:, 12] = d2h
    R[:, 13] = d2m
    R[:, 14] = d2l
    L[:, 15] = s2h
    L[:, 16] = s2m
    L[:, 17] = s2l
    R[:, 15:18] = 1.0
    return L.astype(bf16np), R.astype(bf16np)


# ----------------------------------------------------------------------------
# Cached PJRT runner (compile once, execute many)
# ----------------------------------------------------------------------------

_STATE = {}


def _get_runner():
    if "fn" in _STATE:
        return _STATE

    import jax
    from jax.experimental.shard_map import shard_map
    from jax.sharding import Mesh, PartitionSpec
    from concourse import bass2jax, mybir

    nc = build_nc(M, N)
    bass2jax.install_neuronx_cc_hook()

    in_names, out_names, out_avals = [], [], []
    for alloc in nc.m.functions[0].allocations:
        if not isinstance(alloc, mybir.MemoryLocationSet):
            continue
        name = alloc.memorylocations[0].name
        if alloc.kind == "ExternalInput":
            in_names.append(name)
        elif alloc.kind == "ExternalOutput":
            out_names.append(name)
            out_avals.append(jax.core.ShapedArray(
                tuple(alloc.tensor_shape), mybir.dt.np(alloc.dtype)))
    n_params = len(in_names)
    n_outs = len(out_names)
    all_in_names = tuple(in_names + out_names)
    donate = tuple(range(n_params, n_params + n_outs))

    def _jbody(*args):
        outs = bass2jax._bass_exec_p.bind(
            *args,
            out_avals=tuple(out_avals),
            in_names=all_in_names,
            out_names=tuple(out_names),
            lowering_input_output_aliases=(),
            sim_require_finite=True,
            sim_require_nnan=True,
            nc=nc,
        )
        return tuple(outs)

    devices = jax.devices()[:NCORES]
    mesh = Mesh(np.asarray(devices), ("core",))
    in_specs = (PartitionSpec("core"),) * (n_params + n_outs)
    out_specs = (PartitionSpec("core"),) * n_outs
    fn = jax.jit(
        shard_map(_jbody, mesh=mesh, in_specs=in_specs, out_specs=out_specs,
                  check_rep=False),
        donate_argnums=donate, keep_unused=True,
    )
    _STATE.update(fn=fn, nc=nc, in_names=in_names, out_names=out_names,
                  out_avals=out_avals, n_params=n_params)
    return _STATE


def run_device(L, R):
    """L, R: [NCORES, 18, M] bf16. Returns (rowmin[NCORES,128,M/128],
    colmin[NCORES,32,N/32]) squared-distance minima (fp32)."""
    st = _get_runner()
    concat_in = []
    for name in st["in_names"]:
        arr = L if name == "lhs_aug" else R
        concat_in.append(np.concatenate([arr[c] for c in range(NCORES)], axis=0))
    concat_zeros = [
        np.zeros((NCORES * av.shape[0], *av.shape[1:]), av.dtype)
        for av in st["out_avals"]
    ]
    out_arrs = st["fn"](*concat_in, *concat_zeros)
    outs = {}
    for i, name in enumerate(st["out_names"]):
        av = st["out_avals"][i]
        outs[name] = np.asarray(out_arrs[i]).reshape(NCORES, *av.shape)
    return outs["rowmin"], outs["colmin"]


# ----------------------------------------------------------------------------
# Public entry point
# ----------------------------------------------------------------------------

def _host_reduce(rowmin, colmin):
    # rowmin: [B, 128, M/128]; colmin: [B, 32, N/32]  (squared distances)
    fwd = np.sqrt(np.maximum(rowmin.astype(np.float64), 0.0)).mean()
    bwd = np.sqrt(np.maximum(colmin.astype(np.float64), 0.0)).mean()
    total = np.float32(fwd + bwd)
    return total


def kernel(pc_src, pc_dst):
    L, R = prep_inputs(pc_src, pc_dst)
    rowmin, colmin = run_device(L, R)
    total = _host_reduce(rowmin, colmin)
    return (total, total, total)


# revision 15
# speedup vs baseline: 641.9407x; 641.9407x over previous
"""Chamfer loss (brute-force, no sigma) on 8 trn2 NeuronCores.

Strategy (data-parallel over batch, one batch element per core):
  sq[m,n] = |src_m - dst_n|^2 is produced by ONE augmented matmul per tile:
     sq = L^T @ R,  K = 18 rows:
       rows 0-11 : exact 2-term bf16 split of -2*src_c x dst_c  (hi/lo cross terms)
       rows 12-14: ones (x) 3-term bf16 split of |dst_n|^2
       rows 15-17: 3-term bf16 split of |src_m|^2 (x) ones
  PE accumulates in fp32, so the full squared distance (small, >=0) is formed
  before any rounding.  ScalarE evacuates each PSUM tile to SBUF as bf16;
  VectorE then does a fused (elementwise-min + row-min-reduce) for the
  src->dst direction and a running elementwise min for the dst->src
  direction.  The dst->src partition-axis min is finished with a 32x32
  stream transpose + reduce + two cross-quadrant folds.
  Host side only shards/preps inputs and takes sqrt/mean of the 2*4096
  per-core minima.
"""

import numpy as np
import ml_dtypes
from contextlib import ExitStack

B, C = 8, 3
M = N = 4096
NCORES = 8
PB = 128          # output partition block (m rows per matmul)
KAUG = 18         # augmented contraction dim
BIG = 1.0e4       # > max possible squared distance (~150)
MMN = 512         # matmul moving free dim (one fp32 PSUM bank)

bf16np = ml_dtypes.bfloat16

# Toggles (fallbacks in case an op is unsupported somewhere in the stack)
USE_TTR = True        # fused tensor_tensor_reduce for the forward row-min
XPART_SLICE = False   # walrus rejects TT with mismatched SB base partitions


# ----------------------------------------------------------------------------
# Device program
# ----------------------------------------------------------------------------

def _body(ctx, tc, lhs, rhs, rowmin_d, colmin_d, m, n, reps=1):
    import concourse.mybir as mybir

    nc = tc.nc
    f32 = mybir.dt.float32
    bf16 = mybir.dt.bfloat16
    MIN = mybir.AluOpType.min
    AX = mybir.AxisListType.X

    nblk = m // PB
    half = n // 2
    mmn = MMN if half % MMN == 0 else half
    nq = half // mmn
    nb32 = half // 32

    cpool = ctx.enter_context(tc.tile_pool(name="const", bufs=1))
    ppool = ctx.enter_context(tc.tile_pool(name="psum", bufs=2, space="PSUM"))
    spool = ctx.enter_context(tc.tile_pool(name="sb", bufs=3))
    rpool = ctx.enter_context(tc.tile_pool(name="scr", bufs=2))

    lhs_t = cpool.tile([KAUG, m], bf16)
    nc.sync.dma_start(out=lhs_t[:], in_=lhs[:])
    rhs_t = cpool.tile([KAUG, n], bf16)
    nc.sync.dma_start(out=rhs_t[:], in_=rhs[:])

    rowmin_t = cpool.tile([PB, nblk], f32)
    rm2 = cpool.tile([PB, 2 * nblk], f32)
    baccs = []
    for g in range(2):
        bacc = cpool.tile([PB, half], bf16, tag=f"bacc{g}")
        nc.vector.memset(bacc[:], BIG)
        baccs.append(bacc)

    for rep in range(reps):
      for i in range(nblk):
        for h in range(2):
            pt = ppool.tile([PB, half], f32, tag="pt")
            for q in range(nq):
                nc.tensor.matmul(
                    pt[:, q * mmn:(q + 1) * mmn],
                    lhs_t[:, i * PB:(i + 1) * PB],
                    rhs_t[:, h * half + q * mmn: h * half + (q + 1) * mmn],
                    start=True, stop=True,
                )
            sb = spool.tile([PB, half], bf16, tag="sb")
            nc.scalar.copy(sb[:], pt[:])

            # fused elementwise no-op + free-dim min-reduce into rm2 column
            scr = rpool.tile([PB, half], bf16, tag="scr")
            nc.vector.tensor_scalar(scr[:], sb[:], float(BIG), None, MIN, MIN,
                                    accum_out=rm2[:, 2 * i + h: 2 * i + h + 1])
            nc.vector.tensor_tensor(baccs[h][:], baccs[h][:], sb[:], MIN)

    nc.vector.tensor_reduce(
        rowmin_t[:], rm2[:].rearrange("p (i h) -> p i h", h=2), AX, MIN)
    nc.sync.dma_start(out=rowmin_d[:], in_=rowmin_t[:])

    # dst->src direction: partition-axis min of bacc[p, nn] over p.
    colmin_sb = cpool.tile([32, n // 32], f32)
    for g in range(2):
        tr = rpool.tile([PB, half], bf16, tag="tr")
        nc.vector.transpose(tr[:], baccs[g][:])
        red = rpool.tile([PB, nb32], f32, tag="red")
        nc.vector.tensor_reduce(
            red[:], tr[:].rearrange("p (b i) -> p b i", i=32), AX, MIN)
        if XPART_SLICE:
            t1 = rpool.tile([64, nb32], f32, tag="t1")
            nc.vector.tensor_tensor(t1[:], red[0:64, :], red[64:128, :], MIN)
            nc.vector.tensor_tensor(colmin_sb[:, g * nb32:(g + 1) * nb32],
                                    t1[0:32, :], t1[32:64, :], MIN)
        else:
            hi = rpool.tile([64, nb32], f32, tag="hi")
            nc.sync.dma_start(out=hi[:], in_=red[64:128, :])
            t1 = rpool.tile([64, nb32], f32, tag="t1")
            nc.vector.tensor_tensor(t1[:], red[0:64, :], hi[:], MIN)
            hi2 = rpool.tile([32, nb32], f32, tag="hi2")
            nc.sync.dma_start(out=hi2[:], in_=t1[32:64, :])
            nc.vector.tensor_tensor(colmin_sb[:, g * nb32:(g + 1) * nb32],
                                    t1[0:32, :], hi2[:], MIN)
    nc.sync.dma_start(out=colmin_d[:], in_=colmin_sb[:])


def build_nc(m=M, n=N, reps=1):
    import concourse.tile as tile
    import concourse.bacc as bacc
    import concourse.mybir as mybir

    f32 = mybir.dt.float32
    bf16 = mybir.dt.bfloat16
    nblk = m // PB

    nc = bacc.Bacc("TRN2", target_bir_lowering=False, debug=False)
    lhs = nc.dram_tensor("lhs_aug", [KAUG, m], bf16, kind="ExternalInput").ap()
    rhs = nc.dram_tensor("rhs_aug", [KAUG, n], bf16, kind="ExternalInput").ap()
    rowmin_d = nc.dram_tensor("rowmin", [PB, nblk], f32,
                              kind="ExternalOutput").ap()
    colmin_d = nc.dram_tensor("colmin", [32, n // 32], f32,
                              kind="ExternalOutput").ap()
    with tile.TileContext(nc) as tc:
        with ExitStack() as ctx:
            _body(ctx, tc, lhs, rhs, rowmin_d, colmin_d, m, n, reps=reps)
    nc.compile()
    return nc


# ----------------------------------------------------------------------------
# Host-side input prep: exact bf16 splits for the augmented operands
# ----------------------------------------------------------------------------

def _split2(x):
    """x (f64) -> (hi, lo) bf16 values returned as exact f64."""
    hi = x.astype(bf16np).astype(np.float64)
    lo = (x - hi).astype(bf16np).astype(np.float64)
    return hi, lo


def _split3(x):
    h = x.astype(bf16np).astype(np.float64)
    r = x - h
    mdl = r.astype(bf16np).astype(np.float64)
    l = (r - mdl).astype(bf16np).astype(np.float64)
    return h, mdl, l


def prep_inputs(pc_src, pc_dst):
    """Build per-batch augmented operands L, R: [B, 18, M/N] bf16."""
    s = np.asarray(pc_src, dtype=np.float64)   # [B, 3, M]
    d = np.asarray(pc_dst, dtype=np.float64)   # [B, 3, N]
    b, c, m = s.shape
    n = d.shape[2]

    s_hi, s_lo = _split2(s)
    d_hi, d_lo = _split2(d)
    s_eff = s_hi + s_lo
    d_eff = d_hi + d_lo
    s2 = (s_eff ** 2).sum(axis=1)              # [B, M]
    d2 = (d_eff ** 2).sum(axis=1)              # [B, N]
    s2h, s2m, s2l = _split3(s2)
    d2h, d2m, d2l = _split3(d2)

    L = np.zeros((b, KAUG, m), dtype=np.float64)
    R = np.zeros((b, KAUG, n), dtype=np.float64)
    L[:, 0:3] = -2.0 * s_hi
    R[:, 0:3] = d_hi
    L[:, 3:6] = -2.0 * s_hi
    R[:, 3:6] = d_lo
    L[:, 6:9] = -2.0 * s_lo
    R[:, 6:9] = d_hi
    L[:, 9:12] = -2.0 * s_lo
    R[:, 9:12] = d_lo
    L[:, 12:15] = 1.0
    R[:, 12] = d2h
    R[:, 13] = d2m
    R[:, 14] = d2l
    L[:, 15] = s2h
    L[:, 16] = s2m
    L[:, 17] = s2l
    R[:, 15:18] = 1.0
    return L.astype(bf16np), R.astype(bf16np)


# ----------------------------------------------------------------------------
# Cached PJRT runner (compile once, execute many)
# ----------------------------------------------------------------------------

_STATE = {}


def _get_runner(reps=1):
    if reps in _STATE:
        return _STATE[reps]

    import jax
    from jax.experimental.shard_map import shard_map
    from jax.sharding import Mesh, PartitionSpec
    from concourse import bass2jax, mybir

    nc = build_nc(M, N, reps=reps)
    bass2jax.install_neuronx_cc_hook()

    in_names, out_names, out_avals = [], [], []
    for alloc in nc.m.functions[0].allocations:
        if not isinstance(alloc, mybir.MemoryLocationSet):
            continue
        name = alloc.memorylocations[0].name
        if alloc.kind == "ExternalInput":
            in_names.append(name)
        elif alloc.kind == "ExternalOutput":
            out_names.append(name)
            out_avals.append(jax.core.ShapedArray(
                tuple(alloc.tensor_shape), mybir.dt.np(alloc.dtype)))
    n_params = len(in_names)
    n_outs = len(out_names)
    all_in_names = tuple(in_names + out_names)
    donate = tuple(range(n_params, n_params + n_outs))

    def _jbody(*args):
        outs = bass2jax._bass_exec_p.bind(
            *args,
            out_avals=tuple(out_avals),
            in_names=all_in_names,
            out_names=tuple(out_names),
            lowering_input_output_aliases=(),
            sim_require_finite=True,
            sim_require_nnan=True,
            nc=nc,
        )
        return tuple(outs)

    devices = jax.devices()[:NCORES]
    mesh = Mesh(np.asarray(devices), ("core",))
    in_specs = (PartitionSpec("core"),) * (n_params + n_outs)
    out_specs = (PartitionSpec("core"),) * n_outs
    fn = jax.jit(
        shard_map(_jbody, mesh=mesh, in_specs=in_specs, out_specs=out_specs,
                  check_rep=False),
        donate_argnums=donate, keep_unused=True,
    )
    st = dict(fn=fn, nc=nc, in_names=in_names, out_names=out_names,
              out_avals=out_avals, n_params=n_params)
    _STATE[reps] = st
    return st


def run_device(L, R, reps=1):
    """L, R: [NCORES, 18, M] bf16. Returns (rowmin[NCORES,128,M/128],
    colmin[NCORES,32,N/32]) squared-distance minima (fp32)."""
    st = _get_runner(reps)
    concat_in = []
    for name in st["in_names"]:
        arr = L if name == "lhs_aug" else R
        concat_in.append(np.concatenate([arr[c] for c in range(NCORES)], axis=0))
    concat_zeros = [
        np.zeros((NCORES * av.shape[0], *av.shape[1:]), av.dtype)
        for av in st["out_avals"]
    ]
    out_arrs = st["fn"](*concat_in, *concat_zeros)
    outs = {}
    for i, name in enumerate(st["out_names"]):
        av = st["out_avals"][i]
        outs[name] = np.asarray(out_arrs[i]).reshape(NCORES, *av.shape)
    return outs["rowmin"], outs["colmin"]


# ----------------------------------------------------------------------------
# Public entry point
# ----------------------------------------------------------------------------

def _host_reduce(rowmin, colmin):
    # rowmin: [B, 128, M/128]; colmin: [B, 32, N/32]  (squared distances)
    fwd = np.sqrt(np.maximum(rowmin.astype(np.float64), 0.0)).mean()
    bwd = np.sqrt(np.maximum(colmin.astype(np.float64), 0.0)).mean()
    total = np.float32(fwd + bwd)
    return total


def kernel(pc_src, pc_dst):
    L, R = prep_inputs(pc_src, pc_dst)
    rowmin, colmin = run_device(L, R)
    total = _host_reduce(rowmin, colmin)
    return (total, total, total)
